# revision 10
# baseline (speedup 1.0000x reference)
"""Distributed causal attention (RoPE + QK-RMSNorm) for TRN2, 8 NeuronCores.

Problem: B=2, T=2048, C=2048, NH=16 heads of H=128; y = Attn(x) with
 q/k = RMSNorm(RoPE(x @ W{q,k}.T)), causal SDPA, out proj Wo.

Sharding: tensor-parallel over heads x data-parallel over batch.
core i = (b = i//4, g = i%4) owns batch b and heads [4g, 4g+4).
Wo row-partials are combined with a bf16 ReduceScatter over each batch
group of 4 cores, one RS per 512-token query chunk so comm overlaps
compute; core (b, g) emits output tokens qc*512 + g*128 .. +128.

v2 vs baseline (847us):
- all matmul operands bf16 (same PE rate as f32r at free>=256, but
  halves DMA/SBUF and doubles DVE throughput on elementwise work)
- all transposes via DMA xbar (dma_start_transpose) instead of PE
  identity-matmuls: ~55us PE saved
- x loaded once: Q projection computed in phase 0 alongside K/V
- ReduceScatter payload bf16 (4x less ring traffic), per-chunk
  dedicated DRAM bounce tiles (no WAR stalls), single RS per chunk
- PV accumulation opened with start=(kt==0) instead of zrow matmuls
- QK matmuls stream only the unmasked column range
- engine split: scalar=exp/rsqrt only, gpsimd=copies+masks, vector=
  rope/norm/reduce/softmax-scale
"""
import os
import sys

if "/opt/trn_rl_repo" not in sys.path:
    sys.path.insert(0, "/opt/trn_rl_repo")

import numpy as np
import ml_dtypes

import concourse.bass as bass
import concourse.mybir as mybir
import concourse.tile as tile
from concourse import bacc
from concourse.bass_utils import run_bass_kernel_spmd

B, T, C = 2, 2048, 2048
NH, H = 16, 128
HB = 4           # heads per core
G = 4            # head-groups (= cores per batch)
P = 128
NTT = T // P     # 16 token tiles
QC = 512         # query chunk
NQC = T // QC    # 4 query chunks
TS = QC // P     # 4 token tiles per query chunk
CT = C // P      # 16 contraction tiles
EPS = float(np.finfo(np.float32).eps)

F32 = mybir.dt.float32
BF16 = mybir.dt.bfloat16
AF = mybir.ActivationFunctionType
ALU = mybir.AluOpType

_NC_CACHE = {}


def build_nc(apply_w=False):
    nc = bacc.Bacc("TRN2", target_bir_lowering=False, debug=False,
                   num_devices=8)

    xT = nc.dram_tensor("xT", [C, T], BF16, kind="ExternalInput").ap()
    wq = nc.dram_tensor("wq", [C, HB * H], BF16, kind="ExternalInput").ap()
    wk = nc.dram_tensor("wk", [C, HB * H], BF16, kind="ExternalInput").ap()
    wv = nc.dram_tensor("wv", [C, HB * H], BF16, kind="ExternalInput").ap()
    wo = nc.dram_tensor("wo", [HB * H, C], BF16, kind="ExternalInput").ap()
    cos_e = nc.dram_tensor("cos", [T, H // 2], BF16, kind="ExternalInput").ap()
    sin_e = nc.dram_tensor("sin", [T, H // 2], BF16, kind="ExternalInput").ap()
    mask_e = nc.dram_tensor("mask", [P, P], BF16, kind="ExternalInput").ap()
    qw_e = nc.dram_tensor("qw", [P, HB * H], BF16, kind="ExternalInput").ap()
    kw_e = nc.dram_tensor("kw", [P, HB * H], BF16, kind="ExternalInput").ap()
    out_e = nc.dram_tensor("out", [NQC, P, C], BF16, kind="ExternalOutput").ap()

    with tile.TileContext(nc) as tc:
        with tc.tile_pool(name="const", bufs=1) as cpool, \
             tc.tile_pool(name="wpool", bufs=1) as wpool, \
             tc.tile_pool(name="big", bufs=1) as bigpool, \
             tc.tile_pool(name="xs", bufs=3) as xpool, \
             tc.tile_pool(name="work", bufs=2) as wk_pool, \
             tc.tile_pool(name="qng", bufs=2) as qpool, \
             tc.tile_pool(name="ptile", bufs=6) as ppool, \
             tc.tile_pool(name="obuf", bufs=2) as opool, \
             tc.tile_pool(name="ccdram", bufs=1, space="DRAM") as ccpool, \
             tc.tile_pool(name="psA", bufs=2, space="PSUM") as psA, \
             tc.tile_pool(name="psS", bufs=2, space="PSUM") as psS, \
             tc.tile_pool(name="psO", bufs=4, space="PSUM") as psO:

            # ---- constants ----
            cos_sb = cpool.tile([P, NTT, H // 2], BF16)
            sin_sb = cpool.tile([P, NTT, H // 2], BF16)
            nc.sync.dma_start(cos_sb[:], cos_e.rearrange("(tt p) j -> p tt j", p=P))
            nc.sync.dma_start(sin_sb[:], sin_e.rearrange("(tt p) j -> p tt j", p=P))
            mask_sb = cpool.tile([P, P], BF16)
            nc.sync.dma_start(mask_sb[:], mask_e)
            zrow_sb = cpool.tile([P, 2 * 130], BF16)
            nc.vector.memset(zrow_sb[:], 0.0)
            if apply_w:
                qw_sb = cpool.tile([P, HB * H], BF16)
                kw_sb = cpool.tile([P, HB * H], BF16)
                nc.sync.dma_start(qw_sb[:], qw_e)
                nc.sync.dma_start(kw_sb[:], kw_e)
            epsq_sb = cpool.tile([P, 1], F32)
            epsk_sb = cpool.tile([P, 1], F32)
            nc.vector.memset(epsq_sb[:], float(H) * EPS)
            nc.vector.memset(epsk_sb[:], EPS)

            # ---- persistent big tensors (all bf16) ----
            kT_sb = bigpool.tile([P, NTT, HB, P], BF16)      # [h, kt, hb, tk]
            qT_sb = bigpool.tile([P, HB, NTT, P], BF16)      # [h, hb, qt, tq]
            v_sb = bigpool.tile([P, NTT, HB, H + 1], BF16)   # [tk, kt, hb, h|1]
            nc.vector.memset(v_sb[:, :, :, H:H + 1], 1.0)

            # ---- weights ----
            wk_sb = wpool.tile([P, CT, HB * H], BF16, tag="wk")
            wv_sb = wpool.tile([P, CT, HB * H], BF16, tag="wv")
            wq_sb = wpool.tile([P, CT, HB * H], BF16, tag="wq")
            wo_sb = wpool.tile([P, HB, C], BF16, tag="wo")
            for wdst, wsrc in ((wk_sb, wk), (wv_sb, wv), (wq_sb, wq)):
                half = CT // 2
                wr = wsrc.rearrange("(ct p) h -> p ct h", p=P)
                nc.sync.dma_start(wdst[:, 0:half, :], wr[:, 0:half, :])
                nc.scalar.dma_start(wdst[:, half:, :], wr[:, half:, :])
            nc.scalar.dma_start(wo_sb[:], wo.rearrange("(fb p) c -> p fb c", p=P))

            def proj(x_tile, w_sb, name):
                pp = psA.tile([P, HB, H], F32, tag="proj", name=name)
                for ct in range(CT):
                    nc.tensor.matmul(
                        pp[:].rearrange("p hb h -> p (hb h)"),
                        x_tile[:, ct, :], w_sb[:, ct, :],
                        start=(ct == 0), stop=(ct == CT - 1))
                return pp

            def rope(tt, i, pp, dstG, w_sb, msg):
                """PSUM proj -> bf16 rope -> dstG[:, i]; sumsq -> msg col."""
                q0 = wk_pool.tile([P, HB, H], BF16, tag="q0", name=f"q0_{tt}")
                nc.scalar.copy(q0[:], pp[:])
                cos_b = cos_sb[:, tt, :].unsqueeze(1).broadcast_to([P, HB, H // 2])
                sin_b = sin_sb[:, tt, :].unsqueeze(1).broadcast_to([P, HB, H // 2])
                x1 = q0[:, :, 0:H // 2]
                x2 = q0[:, :, H // 2:H]
                r1 = wk_pool.tile([P, HB, H // 2], BF16, tag="r1", name=f"r1_{tt}")
                r2 = wk_pool.tile([P, HB, H // 2], BF16, tag="r2", name=f"r2_{tt}")
                qn = dstG[:, i]
                nc.vector.tensor_mul(r1[:], x1, cos_b)
                nc.vector.tensor_mul(r2[:], x2, sin_b)
                nc.vector.tensor_sub(qn[:, :, 0:H // 2], r1[:], r2[:])
                nc.vector.tensor_mul(r1[:], x1, sin_b)
                nc.vector.tensor_mul(r2[:], x2, cos_b)
                nc.vector.tensor_add(qn[:, :, H // 2:H], r1[:], r2[:])
                if apply_w:
                    nc.gpsimd.tensor_mul(
                        qn[:].rearrange("p hb h -> p (hb h)"),
                        qn[:].rearrange("p hb h -> p (hb h)"), w_sb[:])
                sq = wk_pool.tile([P, HB, H], F32, tag="sq", name=f"sq_{tt}")
                nc.gpsimd.tensor_mul(sq[:], qn[:], qn[:])
                nc.vector.tensor_reduce(
                    out=msg[:, i * HB:(i + 1) * HB], in_=sq[:], op=ALU.add,
                    axis=mybir.AxisListType.X)

            def norm_scale(knG, qnG, msgk, msgq, gidx):
                """Batched rsqrt over the 4-tile group (one Ln/Exp table-load
                pair covers both k and q), then scale heads."""
                rsk = wk_pool.tile([P, HB * TS], F32, tag="rsk", name=f"rk{gidx}")
                rsq = wk_pool.tile([P, HB * TS], F32, tag="rsq", name=f"rq{gidx}")
                nc.scalar.activation(rsk[:], msgk[:], AF.Ln, bias=epsk_sb[:],
                                     scale=1.0 / H)
                nc.scalar.activation(rsq[:], msgq[:], AF.Ln, bias=epsq_sb[:])
                nc.scalar.activation(rsk[:], rsk[:], AF.Exp, scale=-0.5)
                nc.scalar.activation(rsq[:], rsq[:], AF.Exp, scale=-0.5)
                for i in range(TS):
                    for hb in range(HB):
                        col = slice(i * HB + hb, i * HB + hb + 1)
                        nc.vector.tensor_scalar_mul(
                            knG[:, i, hb, :], knG[:, i, hb, :], rsk[:, col])
                        nc.vector.tensor_scalar_mul(
                            qnG[:, i, hb, :], qnG[:, i, hb, :], rsq[:, col])

            def load_x(tt):
                x_tile = xpool.tile([P, CT, P], BF16, tag="xs", name=f"x{tt}")
                nc.sync.dma_start(
                    x_tile[:],
                    xT[:, tt * P:(tt + 1) * P].rearrange("(ct p) t -> p ct t", p=P))
                return x_tile

            # ======== phase 0: K, V, Q for all tokens ========
            for gg in range(NTT // TS):
                knG = qpool.tile([P, TS, HB, H], BF16, tag="kn", name=f"kn{gg}")
                qnG = qpool.tile([P, TS, HB, H], BF16, tag="qn", name=f"qn{gg}")
                msgk = wk_pool.tile([P, HB * TS], F32, tag="mgk", name=f"mk{gg}")
                msgq = wk_pool.tile([P, HB * TS], F32, tag="mgq", name=f"mq{gg}")
                for i in range(TS):
                    tt = gg * TS + i
                    x_tile = load_x(tt)
                    pk = proj(x_tile, wk_sb, f"pk{tt}")
                    rope(tt, i, pk, knG, kw_sb if apply_w else None, msgk)
                    pv = proj(x_tile, wv_sb, f"pv{tt}")
                    nc.scalar.copy(v_sb[:, tt, :, 0:H], pv[:])
                    pq = proj(x_tile, wq_sb, f"pq{tt}")
                    rope(tt, i, pq, qnG, qw_sb if apply_w else None, msgq)
                norm_scale(knG, qnG, msgk, msgq, gg)
                # batched K transpose: [tok,(i hb h)] -> [h,(kt hb),tok]
                nc.sync.dma_start_transpose(
                    kT_sb[:, gg * TS:(gg + 1) * TS, :, :].rearrange(
                        "p i hb q -> p (i hb) q"),
                    knG[:].rearrange("p i hb h -> p (i hb h)"))
                # per-tile Q transpose: [tok,(hb h)] -> [h, hb, tok]
                for i in range(TS):
                    nc.sync.dma_start_transpose(
                        qT_sb[:, :, gg * TS + i, :],
                        qnG[:, i].rearrange("p hb h -> p (hb h)"))

            # ======== phase 1: attention + Wo + RS per query chunk ========
            for qc in range(NQC):
                a_sb = opool.tile([P, TS, HB * H], BF16, tag="a",
                                  name=f"a{qc}")      # [tq, ts, (hb h)]
                nkt = (qc + 1) * TS
                for hp in range(2):
                    hbs = (2 * hp, 2 * hp + 1)
                    o_ps = {}
                    for hb in hbs:
                        for j in range(2):
                            o = psO.tile([P, 2, 130], F32, tag="o",
                                         name=f"o_{qc}_{hb}_{j}")
                            nc.tensor.matmul(
                                o[:].rearrange("p a b -> p (a b)"),
                                zrow_sb[:, 0:P], zrow_sb[:],
                                start=True, stop=False)
                            o_ps[(hb, j)] = o

                    def emit_pv(kt, hb, p_sb):
                        d = kt - qc * TS
                        for ts in range(max(d, 0), TS):
                            nc.tensor.matmul(
                                o_ps[(hb, ts // 2)][:, ts % 2, 0:H + 1],
                                p_sb[:, ts * P:(ts + 1) * P],
                                v_sb[:, kt, hb, :],
                                start=False,
                                stop=(kt == qc * TS + ts))

                    pend = []
                    for kt in range(nkt):
                        d = kt - qc * TS
                        lo = max(d, 0) * P
                        for hb in hbs:
                            sp = psS if hb == hbs[0] else psA
                            stag = "tp" if hb == hbs[0] else "proj"
                            s_ps = sp.tile([P, QC], F32, tag=stag,
                                           name=f"s_{qc}_{hb}_{kt}")
                            nc.tensor.matmul(
                                s_ps[:, lo:], kT_sb[:, kt, hb, :],
                                qT_sb[:, hb, qc * TS:(qc + 1) * TS, :]
                                .rearrange("p ts t -> p (ts t)")[:, lo:],
                                start=True, stop=True)
                            p_sb = ppool.tile([P, QC], BF16, tag="p",
                                              name=f"p_{qc}_{hb}_{kt}")
                            nc.scalar.activation(p_sb[:, lo:], s_ps[:, lo:],
                                                 AF.Exp)
                            if d >= 0:
                                nc.vector.tensor_mul(
                                    p_sb[:, d * P:(d + 1) * P],
                                    p_sb[:, d * P:(d + 1) * P],
                                    mask_sb[:])
                            pend.append((kt, hb, p_sb))
                        # software pipeline: PV for kt-1 runs while exp(kt)
                        # is still on the scalar engine
                        while pend and pend[0][0] < kt:
                            emit_pv(*pend.pop(0))
                    for item in pend:
                        emit_pv(*item)
                    for hb in hbs:
                        for ts in range(TS):
                            o = o_ps[(hb, ts // 2)]
                            den = wk_pool.tile([P, 1], F32, tag="den",
                                               name=f"dn{qc}_{hb}_{ts}")
                            nc.vector.reciprocal(den[:], o[:, ts % 2, H:H + 1])
                            nc.vector.tensor_scalar_mul(
                                a_sb[:, ts, hb * H:(hb + 1) * H],
                                o[:, ts % 2, 0:H], den[:])

                # batched A transpose: [tq,(ts hb h)] -> [h,(ts fb),tq]
                aT_sb = opool.tile([P, TS, HB, P], BF16, tag="aT",
                                   name=f"aT{qc}")    # [h, ts, fb, tq]
                nc.sync.dma_start_transpose(
                    aT_sb[:].rearrange("p ts fb q -> p (ts fb) q"),
                    a_sb[:].rearrange("p ts f -> p (ts f)"))

                # Wo partials -> bf16 bounce -> ReduceScatter
                bounce = ccpool.tile([QC, C], BF16, tag=f"bounce{qc}",
                                     name=f"bounce{qc}")
                for ts in range(TS):
                    obG = opool.tile([P, C], BF16, tag="ob", name=f"ob{qc}_{ts}")
                    for cc in range(C // QC):
                        wo_ps = psA.tile([P, QC], F32, tag="proj",
                                         name=f"wops{qc}_{ts}_{cc}")
                        for fb in range(HB):
                            nc.tensor.matmul(
                                wo_ps[:], aT_sb[:, ts, fb, :],
                                wo_sb[:, fb, cc * QC:(cc + 1) * QC],
                                start=(fb == 0), stop=(fb == HB - 1))
                        nc.scalar.copy(obG[:, cc * QC:(cc + 1) * QC],
                                       wo_ps[:])
                    nc.sync.dma_start(
                        bounce[ts * P:(ts + 1) * P, :], obG[:])
                red = ccpool.tile([P, C], BF16, tag=f"red{qc}",
                                  name=f"red{qc}")
                nc.gpsimd.collective_compute(
                    "ReduceScatter",
                    ALU.add,
                    ins=[bounce[:].opt()],
                    outs=[red[:].opt()],
                    replica_groups=[[0, 1, 2, 3], [4, 5, 6, 7]],
                )
                nc.sync.dma_start(out_e[qc], red[:])

    nc.compile()
    return nc


def _get_nc(apply_w):
    key = ("nc", apply_w)
    if key not in _NC_CACHE:
        _NC_CACHE[key] = build_nc(apply_w)
    return _NC_CACHE[key]


def make_in_maps(x, sin, cos, Wq, Wk, Wv, Wo, q_norm_w, k_norm_w):
    bf = ml_dtypes.bfloat16
    cos_f = np.ascontiguousarray(cos).astype(bf)
    sin_f = np.ascontiguousarray(sin).astype(bf)
    mask = (np.arange(P)[:, None] <= np.arange(P)[None, :]).astype(bf)
    qw = np.tile(np.asarray(q_norm_w, np.float32)[None, :], (P, HB)).astype(bf)
    kw = np.tile(np.asarray(k_norm_w, np.float32)[None, :], (P, HB)).astype(bf)
    in_maps = []
    for i in range(8):
        b, g = divmod(i, G)
        sl = slice(g * HB * H, (g + 1) * HB * H)
        in_maps.append({
            "xT": np.ascontiguousarray(np.asarray(x[b], np.float32).T).astype(bf),
            "wq": np.ascontiguousarray(np.asarray(Wq, np.float32)[sl, :].T).astype(bf),
            "wk": np.ascontiguousarray(np.asarray(Wk, np.float32)[sl, :].T).astype(bf),
            "wv": np.ascontiguousarray(np.asarray(Wv, np.float32)[sl, :].T).astype(bf),
            "wo": np.ascontiguousarray(np.asarray(Wo, np.float32)[:, sl].T).astype(bf),
            "cos": cos_f, "sin": sin_f, "mask": mask, "qw": qw, "kw": kw,
        })
    return in_maps


def assemble_output(results):
    out = np.empty((B, T, C), np.float32)
    for i in range(8):
        b, g = divmod(i, G)
        r = results[i]["out"]  # [NQC, P, C] bf16
        for qc in range(NQC):
            t0 = qc * QC + g * P
            out[b, t0:t0 + P, :] = r[qc].astype(np.float32)
    return out


def kernel(x, sin, cos, Wq, Wk, Wv, Wo, q_norm_w, k_norm_w):
    apply_w = not (np.allclose(np.asarray(q_norm_w), 1.0)
                   and np.allclose(np.asarray(k_norm_w), 1.0))
    nc = _get_nc(apply_w)
    in_maps = make_in_maps(x, sin, cos, Wq, Wk, Wv, Wo, q_norm_w, k_norm_w)
    res = run_bass_kernel_spmd(nc, in_maps, core_ids=list(range(8)))
    return assemble_output(res.results)


# revision 11
# speedup vs baseline: 1.0444x; 1.0444x over previous
"""Distributed causal attention (RoPE + QK-RMSNorm) for TRN2, 8 NeuronCores.

Problem: B=2, T=2048, C=2048, NH=16 heads of H=128; y = Attn(x) with
 q/k = RMSNorm(RoPE(x @ W{q,k}.T)), causal SDPA, out proj Wo.

Sharding: tensor-parallel over heads x data-parallel over batch.
core i = (b = i//4, g = i%4) owns batch b and heads [4g, 4g+4).
Wo row-partials are combined with a bf16 ReduceScatter over each batch
group of 4 cores, one RS per 512-token query chunk so comm overlaps
compute; core (b, g) emits output tokens qc*512 + g*128 .. +128.

v2 vs baseline (847us):
- all matmul operands bf16 (same PE rate as f32r at free>=256, but
  halves DMA/SBUF and doubles DVE throughput on elementwise work)
- all transposes via DMA xbar (dma_start_transpose) instead of PE
  identity-matmuls: ~55us PE saved
- x loaded once: Q projection computed in phase 0 alongside K/V
- ReduceScatter payload bf16 (4x less ring traffic), per-chunk
  dedicated DRAM bounce tiles (no WAR stalls), single RS per chunk
- PV accumulation opened with start=(kt==0) instead of zrow matmuls
- QK matmuls stream only the unmasked column range
- engine split: scalar=exp/rsqrt only, gpsimd=copies+masks, vector=
  rope/norm/reduce/softmax-scale
"""
import os
import sys

if "/opt/trn_rl_repo" not in sys.path:
    sys.path.insert(0, "/opt/trn_rl_repo")

import numpy as np
import ml_dtypes

import concourse.bass as bass
import concourse.mybir as mybir
import concourse.tile as tile
from concourse import bacc
from concourse.bass_utils import run_bass_kernel_spmd

B, T, C = 2, 2048, 2048
NH, H = 16, 128
HB = 4           # heads per core
G = 4            # head-groups (= cores per batch)
P = 128
NTT = T // P     # 16 token tiles
QC = 512         # query chunk
NQC = T // QC    # 4 query chunks
TS = QC // P     # 4 token tiles per query chunk
CT = C // P      # 16 contraction tiles
EPS = float(np.finfo(np.float32).eps)

F32 = mybir.dt.float32
BF16 = mybir.dt.bfloat16
AF = mybir.ActivationFunctionType
ALU = mybir.AluOpType

_NC_CACHE = {}


def build_nc(apply_w=False):
    nc = bacc.Bacc("TRN2", target_bir_lowering=False, debug=False,
                   num_devices=8)

    xh = nc.dram_tensor("xh", [NTT, P, CT * P], BF16, kind="ExternalInput").ap()
    wq = nc.dram_tensor("wq", [P, CT * (HB * H)], BF16, kind="ExternalInput").ap()
    wk = nc.dram_tensor("wk", [P, CT * (HB * H)], BF16, kind="ExternalInput").ap()
    wv = nc.dram_tensor("wv", [P, CT * (HB * H)], BF16, kind="ExternalInput").ap()
    wo = nc.dram_tensor("wo", [P, HB * C], BF16, kind="ExternalInput").ap()
    cos_e = nc.dram_tensor("cos", [P, NTT * (H // 2)], BF16, kind="ExternalInput").ap()
    sin_e = nc.dram_tensor("sin", [P, NTT * (H // 2)], BF16, kind="ExternalInput").ap()
    mask_e = nc.dram_tensor("mask", [P, P], BF16, kind="ExternalInput").ap()
    qw_e = nc.dram_tensor("qw", [P, HB * H], BF16, kind="ExternalInput").ap()
    kw_e = nc.dram_tensor("kw", [P, HB * H], BF16, kind="ExternalInput").ap()
    out_e = nc.dram_tensor("out", [NQC, P, C], BF16, kind="ExternalOutput").ap()

    with tile.TileContext(nc) as tc:
        with tc.tile_pool(name="const", bufs=1) as cpool, \
             tc.tile_pool(name="wpool", bufs=1) as wpool, \
             tc.tile_pool(name="big", bufs=1) as bigpool, \
             tc.tile_pool(name="xs", bufs=3) as xpool, \
             tc.tile_pool(name="work", bufs=2) as wk_pool, \
             tc.tile_pool(name="qng", bufs=2) as qpool, \
             tc.tile_pool(name="ptile", bufs=6) as ppool, \
             tc.tile_pool(name="obuf", bufs=2) as opool, \
             tc.tile_pool(name="ccdram", bufs=1, space="DRAM") as ccpool, \
             tc.tile_pool(name="psA", bufs=2, space="PSUM") as psA, \
             tc.tile_pool(name="psS", bufs=2, space="PSUM") as psS, \
             tc.tile_pool(name="psO", bufs=4, space="PSUM") as psO:

            # ---- constants ----
            cos_sb = cpool.tile([P, NTT, H // 2], BF16)
            sin_sb = cpool.tile([P, NTT, H // 2], BF16)
            nc.sync.dma_start(cos_sb[:], cos_e.rearrange("p (tt j) -> p tt j", tt=NTT))
            nc.sync.dma_start(sin_sb[:], sin_e.rearrange("p (tt j) -> p tt j", tt=NTT))
            mask_sb = cpool.tile([P, P], BF16)
            nc.sync.dma_start(mask_sb[:], mask_e)
            zrow_sb = cpool.tile([P, 2 * 130], BF16)
            nc.vector.memset(zrow_sb[:], 0.0)
            if apply_w:
                qw_sb = cpool.tile([P, HB * H], BF16)
                kw_sb = cpool.tile([P, HB * H], BF16)
                nc.sync.dma_start(qw_sb[:], qw_e)
                nc.sync.dma_start(kw_sb[:], kw_e)
            epsq_sb = cpool.tile([P, 1], F32)
            epsk_sb = cpool.tile([P, 1], F32)
            nc.vector.memset(epsq_sb[:], float(H) * EPS)
            nc.vector.memset(epsk_sb[:], EPS)

            # ---- persistent big tensors (all bf16) ----
            kT_sb = bigpool.tile([P, NTT, HB, P], BF16)      # [h, kt, hb, tk]
            qT_sb = bigpool.tile([P, HB, NTT, P], BF16)      # [h, hb, qt, tq]
            v_sb = bigpool.tile([P, NTT, HB, H + 1], BF16)   # [tk, kt, hb, h|1]
            nc.vector.memset(v_sb[:, :, :, H:H + 1], 1.0)

            # ---- weights ----
            wk_sb = wpool.tile([P, CT, HB * H], BF16, tag="wk")
            wv_sb = wpool.tile([P, CT, HB * H], BF16, tag="wv")
            wq_sb = wpool.tile([P, CT, HB * H], BF16, tag="wq")
            wo_sb = wpool.tile([P, HB, C], BF16, tag="wo")
            for wdst, wsrc in ((wk_sb, wk), (wv_sb, wv), (wq_sb, wq)):
                half = CT // 2
                wr = wsrc.rearrange("p (ct h) -> p ct h", ct=CT)
                nc.sync.dma_start(wdst[:, 0:half, :], wr[:, 0:half, :])
                nc.scalar.dma_start(wdst[:, half:, :], wr[:, half:, :])
            nc.scalar.dma_start(wo_sb[:],
                                wo.rearrange("p (fb c) -> p fb c", fb=HB))

            def proj(x_tile, w_sb, name):
                pp = psA.tile([P, HB, H], F32, tag="proj", name=name)
                for ct in range(CT):
                    nc.tensor.matmul(
                        pp[:].rearrange("p hb h -> p (hb h)"),
                        x_tile[:, ct, :], w_sb[:, ct, :],
                        start=(ct == 0), stop=(ct == CT - 1))
                return pp

            def rope(tt, i, pp, dstG, w_sb, msg):
                """PSUM proj -> bf16 rope -> dstG[:, i]; sumsq -> msg col."""
                q0 = wk_pool.tile([P, HB, H], BF16, tag="q0", name=f"q0_{tt}")
                nc.scalar.copy(q0[:], pp[:])
                cos_b = cos_sb[:, tt, :].unsqueeze(1).broadcast_to([P, HB, H // 2])
                sin_b = sin_sb[:, tt, :].unsqueeze(1).broadcast_to([P, HB, H // 2])
                x1 = q0[:, :, 0:H // 2]
                x2 = q0[:, :, H // 2:H]
                r1 = wk_pool.tile([P, HB, H // 2], BF16, tag="r1", name=f"r1_{tt}")
                r2 = wk_pool.tile([P, HB, H // 2], BF16, tag="r2", name=f"r2_{tt}")
                qn = dstG[:, i]
                nc.vector.tensor_mul(r1[:], x1, cos_b)
                nc.vector.tensor_mul(r2[:], x2, sin_b)
                nc.vector.tensor_sub(qn[:, :, 0:H // 2], r1[:], r2[:])
                nc.vector.tensor_mul(r1[:], x1, sin_b)
                nc.vector.tensor_mul(r2[:], x2, cos_b)
                nc.vector.tensor_add(qn[:, :, H // 2:H], r1[:], r2[:])
                if apply_w:
                    nc.gpsimd.tensor_mul(
                        qn[:].rearrange("p hb h -> p (hb h)"),
                        qn[:].rearrange("p hb h -> p (hb h)"), w_sb[:])
                sq = wk_pool.tile([P, HB, H], F32, tag="sq", name=f"sq_{tt}")
                nc.gpsimd.tensor_mul(sq[:], qn[:], qn[:])
                nc.vector.tensor_reduce(
                    out=msg[:, i * HB:(i + 1) * HB], in_=sq[:], op=ALU.add,
                    axis=mybir.AxisListType.X)

            def norm_scale(knG, qnG, msgk, msgq, gidx):
                """Batched rsqrt over the 4-tile group (one Ln/Exp table-load
                pair covers both k and q), then scale heads."""
                rsk = wk_pool.tile([P, HB * TS], F32, tag="rsk", name=f"rk{gidx}")
                rsq = wk_pool.tile([P, HB * TS], F32, tag="rsq", name=f"rq{gidx}")
                nc.scalar.activation(rsk[:], msgk[:], AF.Ln, bias=epsk_sb[:],
                                     scale=1.0 / H)
                nc.scalar.activation(rsq[:], msgq[:], AF.Ln, bias=epsq_sb[:])
                nc.scalar.activation(rsk[:], rsk[:], AF.Exp, scale=-0.5)
                nc.scalar.activation(rsq[:], rsq[:], AF.Exp, scale=-0.5)
                for i in range(TS):
                    for hb in range(HB):
                        col = slice(i * HB + hb, i * HB + hb + 1)
                        nc.vector.tensor_scalar_mul(
                            knG[:, i, hb, :], knG[:, i, hb, :], rsk[:, col])
                        nc.vector.tensor_scalar_mul(
                            qnG[:, i, hb, :], qnG[:, i, hb, :], rsq[:, col])

            def load_x(tt):
                x_tile = xpool.tile([P, CT, P], BF16, tag="xs", name=f"x{tt}")
                nc.sync.dma_start(
                    x_tile[:], xh[tt].rearrange("p (ct t) -> p ct t", ct=CT))
                return x_tile

            # ======== phase 0: K, V, Q for all tokens ========
            for gg in range(NTT // TS):
                knG = qpool.tile([P, TS, HB, H], BF16, tag="kn", name=f"kn{gg}")
                qnG = qpool.tile([P, TS, HB, H], BF16, tag="qn", name=f"qn{gg}")
                msgk = wk_pool.tile([P, HB * TS], F32, tag="mgk", name=f"mk{gg}")
                msgq = wk_pool.tile([P, HB * TS], F32, tag="mgq", name=f"mq{gg}")
                for i in range(TS):
                    tt = gg * TS + i
                    x_tile = load_x(tt)
                    pk = proj(x_tile, wk_sb, f"pk{tt}")
                    rope(tt, i, pk, knG, kw_sb if apply_w else None, msgk)
                    pv = proj(x_tile, wv_sb, f"pv{tt}")
                    nc.scalar.copy(v_sb[:, tt, :, 0:H], pv[:])
                    pq = proj(x_tile, wq_sb, f"pq{tt}")
                    rope(tt, i, pq, qnG, qw_sb if apply_w else None, msgq)
                norm_scale(knG, qnG, msgk, msgq, gg)
                # batched K transpose: [tok,(i hb h)] -> [h,(kt hb),tok]
                nc.sync.dma_start_transpose(
                    kT_sb[:, gg * TS:(gg + 1) * TS, :, :].rearrange(
                        "p i hb q -> p (i hb) q"),
                    knG[:].rearrange("p i hb h -> p (i hb h)"))
                # per-tile Q transpose: [tok,(hb h)] -> [h, hb, tok]
                for i in range(TS):
                    nc.sync.dma_start_transpose(
                        qT_sb[:, :, gg * TS + i, :],
                        qnG[:, i].rearrange("p hb h -> p (hb h)"))

            # ======== phase 1: attention + Wo + RS per query chunk ========
            for qc in range(NQC):
                a_sb = opool.tile([P, TS, HB * H], BF16, tag="a",
                                  name=f"a{qc}")      # [tq, ts, (hb h)]
                nkt = (qc + 1) * TS
                for hp in range(2):
                    hbs = (2 * hp, 2 * hp + 1)
                    o_ps = {}
                    for hb in hbs:
                        for j in range(2):
                            o = psO.tile([P, 2, 130], F32, tag="o",
                                         name=f"o_{qc}_{hb}_{j}")
                            nc.tensor.matmul(
                                o[:].rearrange("p a b -> p (a b)"),
                                zrow_sb[:, 0:P], zrow_sb[:],
                                start=True, stop=False)
                            o_ps[(hb, j)] = o

                    def emit_pv(kt, hb, p_sb):
                        d = kt - qc * TS
                        for ts in range(max(d, 0), TS):
                            nc.tensor.matmul(
                                o_ps[(hb, ts // 2)][:, ts % 2, 0:H + 1],
                                p_sb[:, ts * P:(ts + 1) * P],
                                v_sb[:, kt, hb, :],
                                start=False,
                                stop=(kt == qc * TS + ts))

                    pend = []
                    for kt in range(nkt):
                        d = kt - qc * TS
                        lo = max(d, 0) * P
                        for hb in hbs:
                            sp = psS if hb == hbs[0] else psA
                            stag = "tp" if hb == hbs[0] else "proj"
                            s_ps = sp.tile([P, QC], F32, tag=stag,
                                           name=f"s_{qc}_{hb}_{kt}")
                            nc.tensor.matmul(
                                s_ps[:, lo:], kT_sb[:, kt, hb, :],
                                qT_sb[:, hb, qc * TS:(qc + 1) * TS, :]
                                .rearrange("p ts t -> p (ts t)")[:, lo:],
                                start=True, stop=True)
                            p_sb = ppool.tile([P, QC], BF16, tag="p",
                                              name=f"p_{qc}_{hb}_{kt}")
                            nc.scalar.activation(p_sb[:, lo:], s_ps[:, lo:],
                                                 AF.Exp)
                            if d >= 0:
                                nc.vector.tensor_mul(
                                    p_sb[:, d * P:(d + 1) * P],
                                    p_sb[:, d * P:(d + 1) * P],
                                    mask_sb[:])
                            pend.append((kt, hb, p_sb))
                        # software pipeline: PV for kt-1 runs while exp(kt)
                        # is still on the scalar engine
                        while pend and pend[0][0] < kt:
                            emit_pv(*pend.pop(0))
                    for item in pend:
                        emit_pv(*item)
                    for hb in hbs:
                        for ts in range(TS):
                            o = o_ps[(hb, ts // 2)]
                            den = wk_pool.tile([P, 1], F32, tag="den",
                                               name=f"dn{qc}_{hb}_{ts}")
                            nc.vector.reciprocal(den[:], o[:, ts % 2, H:H + 1])
                            nc.vector.tensor_scalar_mul(
                                a_sb[:, ts, hb * H:(hb + 1) * H],
                                o[:, ts % 2, 0:H], den[:])

                # batched A transpose: [tq,(ts hb h)] -> [h,(ts fb),tq]
                aT_sb = opool.tile([P, TS, HB, P], BF16, tag="aT",
                                   name=f"aT{qc}")    # [h, ts, fb, tq]
                nc.sync.dma_start_transpose(
                    aT_sb[:].rearrange("p ts fb q -> p (ts fb) q"),
                    a_sb[:].rearrange("p ts f -> p (ts f)"))

                # Wo partials -> bf16 bounce -> ReduceScatter
                bounce = ccpool.tile([QC, C], BF16, tag=f"bounce{qc}",
                                     name=f"bounce{qc}")
                for ts in range(TS):
                    obG = opool.tile([P, C], BF16, tag="ob", name=f"ob{qc}_{ts}")
                    for cc in range(C // QC):
                        wo_ps = psA.tile([P, QC], F32, tag="proj",
                                         name=f"wops{qc}_{ts}_{cc}")
                        for fb in range(HB):
                            nc.tensor.matmul(
                                wo_ps[:], aT_sb[:, ts, fb, :],
                                wo_sb[:, fb, cc * QC:(cc + 1) * QC],
                                start=(fb == 0), stop=(fb == HB - 1))
                        nc.scalar.copy(obG[:, cc * QC:(cc + 1) * QC],
                                       wo_ps[:])
                    nc.sync.dma_start(
                        bounce[ts * P:(ts + 1) * P, :], obG[:])
                red = ccpool.tile([P, C], BF16, tag=f"red{qc}",
                                  name=f"red{qc}")
                nc.gpsimd.collective_compute(
                    "ReduceScatter",
                    ALU.add,
                    ins=[bounce[:].opt()],
                    outs=[red[:].opt()],
                    replica_groups=[[0, 1, 2, 3], [4, 5, 6, 7]],
                )
                nc.gpsimd.dma_start(out_e[qc], red[:])

    nc.compile()
    return nc


def _get_nc(apply_w):
    key = ("nc", apply_w)
    if key not in _NC_CACHE:
        _NC_CACHE[key] = build_nc(apply_w)
    return _NC_CACHE[key]


def make_in_maps(x, sin, cos, Wq, Wk, Wv, Wo, q_norm_w, k_norm_w):
    bf = ml_dtypes.bfloat16

    def part_major(wT):
        # [C, N] -> [P, CT*N]: row p holds [wT[ct*128+p, :] for ct]
        Cdim, N = wT.shape
        return np.ascontiguousarray(
            wT.reshape(Cdim // P, P, N).transpose(1, 0, 2).reshape(P, -1))

    cos_f = part_major(np.asarray(cos, np.float32).astype(bf))
    sin_f = part_major(np.asarray(sin, np.float32).astype(bf))
    mask = (np.arange(P)[:, None] <= np.arange(P)[None, :]).astype(bf)
    qw = np.tile(np.asarray(q_norm_w, np.float32)[None, :], (P, HB)).astype(bf)
    kw = np.tile(np.asarray(k_norm_w, np.float32)[None, :], (P, HB)).astype(bf)
    in_maps = []
    for i in range(8):
        b, g = divmod(i, G)
        sl = slice(g * HB * H, (g + 1) * HB * H)
        xT = np.asarray(x[b], np.float32).T.astype(bf)   # [C, T]
        # xh[tt, p, ct*128+c] = xT[ct*128+p, tt*128+c]
        xh = np.ascontiguousarray(
            xT.reshape(CT, P, NTT, P).transpose(2, 1, 0, 3).reshape(NTT, P, CT * P))
        in_maps.append({
            "xh": xh,
            "wq": part_major(np.asarray(Wq, np.float32)[sl, :].T.astype(bf)),
            "wk": part_major(np.asarray(Wk, np.float32)[sl, :].T.astype(bf)),
            "wv": part_major(np.asarray(Wv, np.float32)[sl, :].T.astype(bf)),
            "wo": part_major(np.asarray(Wo, np.float32)[:, sl].T.astype(bf)),
            "cos": cos_f, "sin": sin_f, "mask": mask, "qw": qw, "kw": kw,
        })
    return in_maps


def assemble_output(results):
    out = np.empty((B, T, C), np.float32)
    for i in range(8):
        b, g = divmod(i, G)
        r = results[i]["out"]  # [NQC, P, C] bf16
        for qc in range(NQC):
            t0 = qc * QC + g * P
            out[b, t0:t0 + P, :] = r[qc].astype(np.float32)
    return out


def kernel(x, sin, cos, Wq, Wk, Wv, Wo, q_norm_w, k_norm_w):
    apply_w = not (np.allclose(np.asarray(q_norm_w), 1.0)
                   and np.allclose(np.asarray(k_norm_w), 1.0))
    nc = _get_nc(apply_w)
    in_maps = make_in_maps(x, sin, cos, Wq, Wk, Wv, Wo, q_norm_w, k_norm_w)
    res = run_bass_kernel_spmd(nc, in_maps, core_ids=list(range(8)))
    return assemble_output(res.results)


# revision 13
# speedup vs baseline: 1.1347x; 1.0864x over previous
"""Distributed causal attention (RoPE + QK-RMSNorm) for TRN2, 8 NeuronCores.

Problem: B=2, T=2048, C=2048, NH=16 heads of H=128; y = Attn(x) with
 q/k = RMSNorm(RoPE(x @ W{q,k}.T)), causal SDPA, out proj Wo.

Sharding: tensor-parallel over heads x data-parallel over batch.
core i = (b = i//4, g = i%4) owns batch b and heads [4g, 4g+4).
Wo row-partials are combined with a bf16 ReduceScatter over each batch
group of 4 cores, one RS per 512-token query chunk so comm overlaps
compute; core (b, g) emits output tokens qc*512 + g*128 .. +128.

v2 vs baseline (847us):
- all matmul operands bf16 (same PE rate as f32r at free>=256, but
  halves DMA/SBUF and doubles DVE throughput on elementwise work)
- all transposes via DMA xbar (dma_start_transpose) instead of PE
  identity-matmuls: ~55us PE saved
- x loaded once: Q projection computed in phase 0 alongside K/V
- ReduceScatter payload bf16 (4x less ring traffic), per-chunk
  dedicated DRAM bounce tiles (no WAR stalls), single RS per chunk
- PV accumulation opened with start=(kt==0) instead of zrow matmuls
- QK matmuls stream only the unmasked column range
- engine split: scalar=exp/rsqrt only, gpsimd=copies+masks, vector=
  rope/norm/reduce/softmax-scale
"""
import os
import sys

if "/opt/trn_rl_repo" not in sys.path:
    sys.path.insert(0, "/opt/trn_rl_repo")

import numpy as np
import ml_dtypes

import concourse.bass as bass
import concourse.mybir as mybir
import concourse.tile as tile
from concourse import bacc
from concourse.bass_utils import run_bass_kernel_spmd
from concourse.masks import make_identity

B, T, C = 2, 2048, 2048
NH, H = 16, 128
HB = 4           # heads per core
G = 4            # head-groups (= cores per batch)
P = 128
NTT = T // P     # 16 token tiles
QC = 512         # query chunk
NQC = T // QC    # 4 query chunks
TS = QC // P     # 4 token tiles per query chunk
CT = C // P      # 16 contraction tiles
EPS = float(np.finfo(np.float32).eps)

F32 = mybir.dt.float32
BF16 = mybir.dt.bfloat16
AF = mybir.ActivationFunctionType
ALU = mybir.AluOpType

_NC_CACHE = {}


def build_nc(apply_w=False):
    nc = bacc.Bacc("TRN2", target_bir_lowering=False, debug=False,
                   num_devices=8)

    xh = nc.dram_tensor("xh", [NTT, P, CT * P], BF16, kind="ExternalInput").ap()
    wq = nc.dram_tensor("wq", [P, CT * (HB * H)], BF16, kind="ExternalInput").ap()
    wk = nc.dram_tensor("wk", [P, CT * (HB * H)], BF16, kind="ExternalInput").ap()
    wv = nc.dram_tensor("wv", [P, CT * (HB * H)], BF16, kind="ExternalInput").ap()
    wo = nc.dram_tensor("wo", [P, HB * C], BF16, kind="ExternalInput").ap()
    cos_e = nc.dram_tensor("cos", [P, NTT * (H // 2)], BF16, kind="ExternalInput").ap()
    sin_e = nc.dram_tensor("sin", [P, NTT * (H // 2)], BF16, kind="ExternalInput").ap()
    mask_e = nc.dram_tensor("mask", [P, P], BF16, kind="ExternalInput").ap()
    qw_e = nc.dram_tensor("qw", [P, HB * H], BF16, kind="ExternalInput").ap()
    kw_e = nc.dram_tensor("kw", [P, HB * H], BF16, kind="ExternalInput").ap()
    out_e = nc.dram_tensor("out", [NQC, P, C], BF16, kind="ExternalOutput").ap()

    with tile.TileContext(nc) as tc:
        with tc.tile_pool(name="const", bufs=1) as cpool, \
             tc.tile_pool(name="wpool", bufs=1) as wpool, \
             tc.tile_pool(name="big", bufs=1) as bigpool, \
             tc.tile_pool(name="xs", bufs=3) as xpool, \
             tc.tile_pool(name="work", bufs=2) as wk_pool, \
             tc.tile_pool(name="qng", bufs=2) as qpool, \
             tc.tile_pool(name="ptile", bufs=6) as ppool, \
             tc.tile_pool(name="obuf", bufs=2) as opool, \
             tc.tile_pool(name="ccdram", bufs=1, space="DRAM") as ccpool, \
             tc.tile_pool(name="psA", bufs=2, space="PSUM") as psA, \
             tc.tile_pool(name="psS", bufs=2, space="PSUM") as psS, \
             tc.tile_pool(name="psO", bufs=4, space="PSUM") as psO:

            # ---- constants ----
            cos_sb = cpool.tile([P, NTT, H // 2], BF16)
            sin_sb = cpool.tile([P, NTT, H // 2], BF16)
            nc.sync.dma_start(cos_sb[:], cos_e.rearrange("p (tt j) -> p tt j", tt=NTT))
            nc.sync.dma_start(sin_sb[:], sin_e.rearrange("p (tt j) -> p tt j", tt=NTT))
            mask_sb = cpool.tile([P, P], BF16)
            nc.sync.dma_start(mask_sb[:], mask_e)
            zrow_sb = cpool.tile([P, 2 * 130], BF16)
            nc.vector.memset(zrow_sb[:], 0.0)
            ident = cpool.tile([P, P], BF16)
            make_identity(nc, ident[:])
            if apply_w:
                qw_sb = cpool.tile([P, HB * H], BF16)
                kw_sb = cpool.tile([P, HB * H], BF16)
                nc.sync.dma_start(qw_sb[:], qw_e)
                nc.sync.dma_start(kw_sb[:], kw_e)
            epsq_sb = cpool.tile([P, 1], F32)
            epsk_sb = cpool.tile([P, 1], F32)
            nc.vector.memset(epsq_sb[:], float(H) * EPS)
            nc.vector.memset(epsk_sb[:], EPS)

            # ---- persistent big tensors (all bf16) ----
            kT_sb = bigpool.tile([P, NTT, HB, P], BF16)      # [h, kt, hb, tk]
            qT_sb = bigpool.tile([P, HB, NTT, P], BF16)      # [h, hb, qt, tq]
            v_sb = bigpool.tile([P, NTT, HB, H + 1], BF16)   # [tk, kt, hb, h|1]
            nc.vector.memset(v_sb[:, :, :, H:H + 1], 1.0)

            # ---- weights ----
            wk_sb = wpool.tile([P, CT, HB * H], BF16, tag="wk")
            wv_sb = wpool.tile([P, CT, HB * H], BF16, tag="wv")
            wq_sb = wpool.tile([P, CT, HB * H], BF16, tag="wq")
            wo_sb = wpool.tile([P, HB, C], BF16, tag="wo")
            for wdst, wsrc in ((wk_sb, wk), (wv_sb, wv), (wq_sb, wq)):
                half = CT // 2
                wr = wsrc.rearrange("p (ct h) -> p ct h", ct=CT)
                nc.sync.dma_start(wdst[:, 0:half, :], wr[:, 0:half, :])
                nc.scalar.dma_start(wdst[:, half:, :], wr[:, half:, :])
            nc.scalar.dma_start(wo_sb[:],
                                wo.rearrange("p (fb c) -> p fb c", fb=HB))

            def proj(x_tile, w_sb, name):
                pp = psA.tile([P, HB, H], F32, tag="proj", name=name)
                for ct in range(CT):
                    nc.tensor.matmul(
                        pp[:].rearrange("p hb h -> p (hb h)"),
                        x_tile[:, ct, :], w_sb[:, ct, :],
                        start=(ct == 0), stop=(ct == CT - 1))
                return pp

            def rope(tt, i, pp, dstG, w_sb, msg):
                """PSUM proj -> bf16 rope -> dstG[:, i]; sumsq -> msg col."""
                q0 = wk_pool.tile([P, HB, H], BF16, tag="q0", name=f"q0_{tt}")
                nc.scalar.copy(q0[:], pp[:])
                cos_b = cos_sb[:, tt, :].unsqueeze(1).broadcast_to([P, HB, H // 2])
                sin_b = sin_sb[:, tt, :].unsqueeze(1).broadcast_to([P, HB, H // 2])
                x1 = q0[:, :, 0:H // 2]
                x2 = q0[:, :, H // 2:H]
                r1 = wk_pool.tile([P, HB, H // 2], BF16, tag="r1", name=f"r1_{tt}")
                r2 = wk_pool.tile([P, HB, H // 2], BF16, tag="r2", name=f"r2_{tt}")
                qn = dstG[:, i]
                nc.vector.tensor_mul(r1[:], x1, cos_b)
                nc.vector.tensor_mul(r2[:], x2, sin_b)
                nc.vector.tensor_sub(qn[:, :, 0:H // 2], r1[:], r2[:])
                nc.vector.tensor_mul(r1[:], x1, sin_b)
                nc.vector.tensor_mul(r2[:], x2, cos_b)
                nc.vector.tensor_add(qn[:, :, H // 2:H], r1[:], r2[:])
                if apply_w:
                    nc.gpsimd.tensor_mul(
                        qn[:].rearrange("p hb h -> p (hb h)"),
                        qn[:].rearrange("p hb h -> p (hb h)"), w_sb[:])
                sq = wk_pool.tile([P, HB, H], F32, tag="sq", name=f"sq_{tt}")
                nc.gpsimd.tensor_mul(sq[:], qn[:], qn[:])
                nc.vector.tensor_reduce(
                    out=msg[:, i * HB:(i + 1) * HB], in_=sq[:], op=ALU.add,
                    axis=mybir.AxisListType.X)

            def norm_scale(knG, qnG, msgk, msgq, gidx):
                """Batched rsqrt over the 4-tile group (one Ln/Exp table-load
                pair covers both k and q), then scale heads."""
                rsk = wk_pool.tile([P, HB * TS], F32, tag="rsk", name=f"rk{gidx}")
                rsq = wk_pool.tile([P, HB * TS], F32, tag="rsq", name=f"rq{gidx}")
                nc.scalar.activation(rsk[:], msgk[:], AF.Ln, bias=epsk_sb[:],
                                     scale=1.0 / H)
                nc.scalar.activation(rsq[:], msgq[:], AF.Ln, bias=epsq_sb[:])
                nc.scalar.activation(rsk[:], rsk[:], AF.Exp, scale=-0.5)
                nc.scalar.activation(rsq[:], rsq[:], AF.Exp, scale=-0.5)
                for i in range(TS):
                    for hb in range(HB):
                        col = slice(i * HB + hb, i * HB + hb + 1)
                        nc.vector.tensor_scalar_mul(
                            knG[:, i, hb, :], knG[:, i, hb, :], rsk[:, col])
                        nc.vector.tensor_scalar_mul(
                            qnG[:, i, hb, :], qnG[:, i, hb, :], rsq[:, col])

            def load_x(tt):
                x_tile = xpool.tile([P, CT, P], BF16, tag="xs", name=f"x{tt}")
                nc.sync.dma_start(
                    x_tile[:], xh[tt].rearrange("p (ct t) -> p ct t", ct=CT))
                return x_tile

            # ======== phase 0: K, V, Q for all tokens ========
            for gg in range(NTT // TS):
                knG = qpool.tile([P, TS, HB, H], BF16, tag="kn", name=f"kn{gg}")
                qnG = qpool.tile([P, TS, HB, H], BF16, tag="qn", name=f"qn{gg}")
                msgk = wk_pool.tile([P, HB * TS], F32, tag="mgk", name=f"mk{gg}")
                msgq = wk_pool.tile([P, HB * TS], F32, tag="mgq", name=f"mq{gg}")
                for i in range(TS):
                    tt = gg * TS + i
                    x_tile = load_x(tt)
                    pk = proj(x_tile, wk_sb, f"pk{tt}")
                    rope(tt, i, pk, knG, kw_sb if apply_w else None, msgk)
                    pv = proj(x_tile, wv_sb, f"pv{tt}")
                    nc.scalar.copy(v_sb[:, tt, :, 0:H], pv[:])
                    pq = proj(x_tile, wq_sb, f"pq{tt}")
                    rope(tt, i, pq, qnG, qw_sb if apply_w else None, msgq)
                norm_scale(knG, qnG, msgk, msgq, gg)
                # batched K transpose: [tok,(i hb h)] -> [h,(kt hb),tok]
                nc.sync.dma_start_transpose(
                    kT_sb[:, gg * TS:(gg + 1) * TS, :, :].rearrange(
                        "p i hb q -> p (i hb) q"),
                    knG[:].rearrange("p i hb h -> p (i hb h)"))
                # per-tile Q transpose: [tok,(hb h)] -> [h, hb, tok]
                for i in range(TS):
                    nc.sync.dma_start_transpose(
                        qT_sb[:, :, gg * TS + i, :],
                        qnG[:, i].rearrange("p hb h -> p (hb h)"))

            # ======== phase 1: attention + Wo + RS per query chunk ========
            for qc in range(NQC):
                a_sb = opool.tile([P, TS, HB * H], BF16, tag="a",
                                  name=f"a{qc}")      # [tq, ts, (hb h)]
                nkt = (qc + 1) * TS
                for hp in range(2):
                    hbs = (2 * hp, 2 * hp + 1)
                    o_ps = {}
                    for hb in hbs:
                        for j in range(2):
                            o = psO.tile([P, 2, 130], F32, tag="o",
                                         name=f"o_{qc}_{hb}_{j}")
                            nc.tensor.matmul(
                                o[:].rearrange("p a b -> p (a b)"),
                                zrow_sb[:, 0:P], zrow_sb[:],
                                start=True, stop=False)
                            o_ps[(hb, j)] = o

                    def emit_pv(kt, hb, p_sb):
                        d = kt - qc * TS
                        for ts in range(max(d, 0), TS):
                            nc.tensor.matmul(
                                o_ps[(hb, ts // 2)][:, ts % 2, 0:H + 1],
                                p_sb[:, ts * P:(ts + 1) * P],
                                v_sb[:, kt, hb, :],
                                start=False,
                                stop=(kt == qc * TS + ts))

                    pend = []
                    for kt in range(nkt):
                        d = kt - qc * TS
                        lo = max(d, 0) * P
                        for hb in hbs:
                            sp = psS if hb == hbs[0] else psA
                            stag = "tp" if hb == hbs[0] else "proj"
                            s_ps = sp.tile([P, QC], F32, tag=stag,
                                           name=f"s_{qc}_{hb}_{kt}")
                            nc.tensor.matmul(
                                s_ps[:, lo:], kT_sb[:, kt, hb, :],
                                qT_sb[:, hb, qc * TS:(qc + 1) * TS, :]
                                .rearrange("p ts t -> p (ts t)")[:, lo:],
                                start=True, stop=True)
                            p_sb = ppool.tile([P, QC], BF16, tag="p",
                                              name=f"p_{qc}_{hb}_{kt}")
                            nc.scalar.activation(p_sb[:, lo:], s_ps[:, lo:],
                                                 AF.Exp)
                            if d >= 0:
                                nc.vector.tensor_mul(
                                    p_sb[:, d * P:(d + 1) * P],
                                    p_sb[:, d * P:(d + 1) * P],
                                    mask_sb[:])
                            pend.append((kt, hb, p_sb))
                        # software pipeline: PV for kt-1 runs while exp(kt)
                        # is still on the scalar engine
                        while pend and pend[0][0] < kt:
                            emit_pv(*pend.pop(0))
                    for item in pend:
                        emit_pv(*item)
                    for hb in hbs:
                        for ts in range(TS):
                            o = o_ps[(hb, ts // 2)]
                            den = wk_pool.tile([P, 1], F32, tag="den",
                                               name=f"dn{qc}_{hb}_{ts}")
                            nc.vector.reciprocal(den[:], o[:, ts % 2, H:H + 1])
                            nc.vector.tensor_scalar_mul(
                                a_sb[:, ts, hb * H:(hb + 1) * H],
                                o[:, ts % 2, 0:H], den[:])

                # A transpose on PE (DMA transposes serialize with
                # collectives, stalling the pipeline on the previous RS)
                aT_sb = opool.tile([P, TS, HB, P], BF16, tag="aT",
                                   name=f"aT{qc}")    # [h, ts, fb, tq]
                for ts in range(TS):
                    tp = psS.tile([P, HB, P], BF16, tag="tp",
                                  name=f"tp{qc}_{ts}")
                    for fb in range(HB):
                        nc.tensor.transpose(
                            tp[:, fb, :], a_sb[:, ts, fb * P:(fb + 1) * P],
                            ident[:])
                    nc.vector.tensor_copy(aT_sb[:, ts], tp[:])

                # Wo partials -> bf16 bounce -> ReduceScatter
                bounce = ccpool.tile([QC, C], BF16, tag=f"bounce{qc}",
                                     name=f"bounce{qc}")
                for ts in range(TS):
                    obG = opool.tile([P, C], BF16, tag="ob", name=f"ob{qc}_{ts}")
                    for cc in range(C // QC):
                        wo_ps = psA.tile([P, QC], F32, tag="proj",
                                         name=f"wops{qc}_{ts}_{cc}")
                        for fb in range(HB):
                            nc.tensor.matmul(
                                wo_ps[:], aT_sb[:, ts, fb, :],
                                wo_sb[:, fb, cc * QC:(cc + 1) * QC],
                                start=(fb == 0), stop=(fb == HB - 1))
                        nc.scalar.copy(obG[:, cc * QC:(cc + 1) * QC],
                                       wo_ps[:])
                    nc.sync.dma_start(
                        bounce[ts * P:(ts + 1) * P, :], obG[:])
                red = ccpool.tile([P, C], BF16, tag=f"red{qc}",
                                  name=f"red{qc}")
                nc.gpsimd.collective_compute(
                    "ReduceScatter",
                    ALU.add,
                    ins=[bounce[:].opt()],
                    outs=[red[:].opt()],
                    replica_groups=[[0, 1, 2, 3], [4, 5, 6, 7]],
                )
                nc.gpsimd.dma_start(out_e[qc], red[:])

    nc.compile()
    return nc


def _get_nc(apply_w):
    key = ("nc", apply_w)
    if key not in _NC_CACHE:
        _NC_CACHE[key] = build_nc(apply_w)
    return _NC_CACHE[key]


def make_in_maps(x, sin, cos, Wq, Wk, Wv, Wo, q_norm_w, k_norm_w):
    bf = ml_dtypes.bfloat16

    def part_major(wT):
        # [C, N] -> [P, CT*N]: row p holds [wT[ct*128+p, :] for ct]
        Cdim, N = wT.shape
        return np.ascontiguousarray(
            wT.reshape(Cdim // P, P, N).transpose(1, 0, 2).reshape(P, -1))

    cos_f = part_major(np.asarray(cos, np.float32).astype(bf))
    sin_f = part_major(np.asarray(sin, np.float32).astype(bf))
    mask = (np.arange(P)[:, None] <= np.arange(P)[None, :]).astype(bf)
    qw = np.tile(np.asarray(q_norm_w, np.float32)[None, :], (P, HB)).astype(bf)
    kw = np.tile(np.asarray(k_norm_w, np.float32)[None, :], (P, HB)).astype(bf)
    in_maps = []
    for i in range(8):
        b, g = divmod(i, G)
        sl = slice(g * HB * H, (g + 1) * HB * H)
        xT = np.asarray(x[b], np.float32).T.astype(bf)   # [C, T]
        # xh[tt, p, ct*128+c] = xT[ct*128+p, tt*128+c]
        xh = np.ascontiguousarray(
            xT.reshape(CT, P, NTT, P).transpose(2, 1, 0, 3).reshape(NTT, P, CT * P))
        in_maps.append({
            "xh": xh,
            "wq": part_major(np.asarray(Wq, np.float32)[sl, :].T.astype(bf)),
            "wk": part_major(np.asarray(Wk, np.float32)[sl, :].T.astype(bf)),
            "wv": part_major(np.asarray(Wv, np.float32)[sl, :].T.astype(bf)),
            "wo": part_major(np.asarray(Wo, np.float32)[:, sl].T.astype(bf)),
            "cos": cos_f, "sin": sin_f, "mask": mask, "qw": qw, "kw": kw,
        })
    return in_maps


def assemble_output(results):
    out = np.empty((B, T, C), np.float32)
    for i in range(8):
        b, g = divmod(i, G)
        r = results[i]["out"]  # [NQC, P, C] bf16
        for qc in range(NQC):
            t0 = qc * QC + g * P
            out[b, t0:t0 + P, :] = r[qc].astype(np.float32)
    return out


def kernel(x, sin, cos, Wq, Wk, Wv, Wo, q_norm_w, k_norm_w):
    apply_w = not (np.allclose(np.asarray(q_norm_w), 1.0)
                   and np.allclose(np.asarray(k_norm_w), 1.0))
    nc = _get_nc(apply_w)
    in_maps = make_in_maps(x, sin, cos, Wq, Wk, Wv, Wo, q_norm_w, k_norm_w)
    res = run_bass_kernel_spmd(nc, in_maps, core_ids=list(range(8)))
    return assemble_output(res.results)


# revision 14
# speedup vs baseline: 1.2041x; 1.0612x over previous
"""Distributed causal attention (RoPE + QK-RMSNorm) for TRN2, 8 NeuronCores.

Problem: B=2, T=2048, C=2048, NH=16 heads of H=128; y = Attn(x) with
 q/k = RMSNorm(RoPE(x @ W{q,k}.T)), causal SDPA, out proj Wo.

Sharding: tensor-parallel over heads x data-parallel over batch.
core i = (b = i//4, g = i%4) owns batch b and heads [4g, 4g+4).
Wo row-partials are combined with a bf16 ReduceScatter over each batch
group of 4 cores, one RS per 512-token query chunk so comm overlaps
compute; core (b, g) emits output tokens qc*512 + g*128 .. +128.

v2 vs baseline (847us):
- all matmul operands bf16 (same PE rate as f32r at free>=256, but
  halves DMA/SBUF and doubles DVE throughput on elementwise work)
- all transposes via DMA xbar (dma_start_transpose) instead of PE
  identity-matmuls: ~55us PE saved
- x loaded once: Q projection computed in phase 0 alongside K/V
- ReduceScatter payload bf16 (4x less ring traffic), per-chunk
  dedicated DRAM bounce tiles (no WAR stalls), single RS per chunk
- PV accumulation opened with start=(kt==0) instead of zrow matmuls
- QK matmuls stream only the unmasked column range
- engine split: scalar=exp/rsqrt only, gpsimd=copies+masks, vector=
  rope/norm/reduce/softmax-scale
"""
import os
import sys

if "/opt/trn_rl_repo" not in sys.path:
    sys.path.insert(0, "/opt/trn_rl_repo")

import numpy as np
import ml_dtypes

import concourse.bass as bass
import concourse.mybir as mybir
import concourse.tile as tile
from concourse import bacc
from concourse.bass_utils import run_bass_kernel_spmd
from concourse.masks import make_identity

B, T, C = 2, 2048, 2048
NH, H = 16, 128
HB = 4           # heads per core
G = 4            # head-groups (= cores per batch)
P = 128
NTT = T // P     # 16 token tiles
QC = 512         # query chunk
NQC = T // QC    # 4 query chunks
TS = QC // P     # 4 token tiles per query chunk
CT = C // P      # 16 contraction tiles
EPS = float(np.finfo(np.float32).eps)

F32 = mybir.dt.float32
BF16 = mybir.dt.bfloat16
AF = mybir.ActivationFunctionType
ALU = mybir.AluOpType

_NC_CACHE = {}


def build_nc(apply_w=False):
    nc = bacc.Bacc("TRN2", target_bir_lowering=False, debug=False,
                   num_devices=8)

    xh = nc.dram_tensor("xh", [NTT, P, CT * P], BF16, kind="ExternalInput").ap()
    wq = nc.dram_tensor("wq", [P, CT * (HB * H)], BF16, kind="ExternalInput").ap()
    wk = nc.dram_tensor("wk", [P, CT * (HB * H)], BF16, kind="ExternalInput").ap()
    wv = nc.dram_tensor("wv", [P, CT * (HB * H)], BF16, kind="ExternalInput").ap()
    wo = nc.dram_tensor("wo", [P, HB * C], BF16, kind="ExternalInput").ap()
    cos_e = nc.dram_tensor("cos", [P, NTT * (H // 2)], BF16, kind="ExternalInput").ap()
    sin_e = nc.dram_tensor("sin", [P, NTT * (H // 2)], BF16, kind="ExternalInput").ap()
    mask_e = nc.dram_tensor("mask", [P, P], BF16, kind="ExternalInput").ap()
    qw_e = nc.dram_tensor("qw", [P, HB * H], BF16, kind="ExternalInput").ap()
    kw_e = nc.dram_tensor("kw", [P, HB * H], BF16, kind="ExternalInput").ap()
    out_e = nc.dram_tensor("out", [NQC, P, C], BF16, kind="ExternalOutput").ap()

    with tile.TileContext(nc) as tc:
        with tc.tile_pool(name="const", bufs=1) as cpool, \
             tc.tile_pool(name="wpool", bufs=1) as wpool, \
             tc.tile_pool(name="big", bufs=1) as bigpool, \
             tc.tile_pool(name="xs", bufs=6) as xpool, \
             tc.tile_pool(name="work", bufs=2) as wk_pool, \
             tc.tile_pool(name="qng", bufs=2) as qpool, \
             tc.tile_pool(name="ptile", bufs=6) as ppool, \
             tc.tile_pool(name="obuf", bufs=2) as opool, \
             tc.tile_pool(name="ccdram", bufs=1, space="DRAM") as ccpool, \
             tc.tile_pool(name="psA", bufs=2, space="PSUM") as psA, \
             tc.tile_pool(name="psS", bufs=2, space="PSUM") as psS, \
             tc.tile_pool(name="psO", bufs=4, space="PSUM") as psO:

            # ---- constants ----
            cos_sb = cpool.tile([P, NTT, H // 2], BF16)
            sin_sb = cpool.tile([P, NTT, H // 2], BF16)
            nc.scalar.dma_start(cos_sb[:], cos_e.rearrange("p (tt j) -> p tt j", tt=NTT))
            nc.scalar.dma_start(sin_sb[:], sin_e.rearrange("p (tt j) -> p tt j", tt=NTT))
            mask_sb = cpool.tile([P, P], BF16)
            nc.scalar.dma_start(mask_sb[:], mask_e)
            zrow_sb = cpool.tile([P, 2 * 130], BF16)
            nc.vector.memset(zrow_sb[:], 0.0)
            ident = cpool.tile([P, P], BF16)
            make_identity(nc, ident[:])
            if apply_w:
                qw_sb = cpool.tile([P, HB * H], BF16)
                kw_sb = cpool.tile([P, HB * H], BF16)
                nc.sync.dma_start(qw_sb[:], qw_e)
                nc.sync.dma_start(kw_sb[:], kw_e)
            epsq_sb = cpool.tile([P, 1], F32)
            epsk_sb = cpool.tile([P, 1], F32)
            nc.vector.memset(epsq_sb[:], float(H) * EPS)
            nc.vector.memset(epsk_sb[:], EPS)

            # ---- persistent big tensors (all bf16) ----
            kT_sb = bigpool.tile([P, NTT, HB, P], BF16)      # [h, kt, hb, tk]
            qT_sb = bigpool.tile([P, HB, NTT, P], BF16)      # [h, hb, qt, tq]
            v_sb = bigpool.tile([P, NTT, HB, H + 1], BF16)   # [tk, kt, hb, h|1]
            nc.vector.memset(v_sb[:, :, :, H:H + 1], 1.0)

            # ---- weights ----
            wk_sb = wpool.tile([P, CT, HB * H], BF16, tag="wk")
            wv_sb = wpool.tile([P, CT, HB * H], BF16, tag="wv")
            wq_sb = wpool.tile([P, CT, HB * H], BF16, tag="wq")
            wo_sb = wpool.tile([P, HB, C], BF16, tag="wo")
            for wdst, wsrc in ((wk_sb, wk), (wv_sb, wv), (wq_sb, wq)):
                half = CT // 2
                wr = wsrc.rearrange("p (ct h) -> p ct h", ct=CT)
                nc.sync.dma_start(wdst[:, 0:half, :], wr[:, 0:half, :])
                nc.scalar.dma_start(wdst[:, half:, :], wr[:, half:, :])
            nc.scalar.dma_start(wo_sb[:],
                                wo.rearrange("p (fb c) -> p fb c", fb=HB))

            def proj(x_tile, w_sb, name):
                pp = psA.tile([P, HB, H], F32, tag="proj", name=name)
                for ct in range(CT):
                    nc.tensor.matmul(
                        pp[:].rearrange("p hb h -> p (hb h)"),
                        x_tile[:, ct, :], w_sb[:, ct, :],
                        start=(ct == 0), stop=(ct == CT - 1))
                return pp

            def rope(tt, i, pp, dstG, w_sb, msg):
                """PSUM proj -> bf16 rope -> dstG[:, i]; sumsq -> msg col."""
                q0 = wk_pool.tile([P, HB, H], BF16, tag="q0", name=f"q0_{tt}")
                nc.vector.tensor_copy(q0[:], pp[:])
                cos_b = cos_sb[:, tt, :].unsqueeze(1).broadcast_to([P, HB, H // 2])
                sin_b = sin_sb[:, tt, :].unsqueeze(1).broadcast_to([P, HB, H // 2])
                x1 = q0[:, :, 0:H // 2]
                x2 = q0[:, :, H // 2:H]
                r1 = wk_pool.tile([P, HB, H // 2], BF16, tag="r1", name=f"r1_{tt}")
                r2 = wk_pool.tile([P, HB, H // 2], BF16, tag="r2", name=f"r2_{tt}")
                qn = dstG[:, i]
                nc.vector.tensor_mul(r1[:], x1, cos_b)
                nc.vector.tensor_mul(r2[:], x2, sin_b)
                nc.vector.tensor_sub(qn[:, :, 0:H // 2], r1[:], r2[:])
                nc.vector.tensor_mul(r1[:], x1, sin_b)
                nc.vector.tensor_mul(r2[:], x2, cos_b)
                nc.vector.tensor_add(qn[:, :, H // 2:H], r1[:], r2[:])
                if apply_w:
                    nc.gpsimd.tensor_mul(
                        qn[:].rearrange("p hb h -> p (hb h)"),
                        qn[:].rearrange("p hb h -> p (hb h)"), w_sb[:])
                sq = wk_pool.tile([P, HB, H], F32, tag="sq", name=f"sq_{tt}")
                nc.gpsimd.tensor_mul(sq[:], qn[:], qn[:])
                nc.vector.tensor_reduce(
                    out=msg[:, i * HB:(i + 1) * HB], in_=sq[:], op=ALU.add,
                    axis=mybir.AxisListType.X)

            def norm_scale(knG, qnG, msgk, msgq, gidx):
                """Batched rsqrt over the 4-tile group (one Ln/Exp table-load
                pair covers both k and q), then scale heads."""
                rsk = wk_pool.tile([P, HB * TS], F32, tag="rsk", name=f"rk{gidx}")
                rsq = wk_pool.tile([P, HB * TS], F32, tag="rsq", name=f"rq{gidx}")
                nc.scalar.activation(rsk[:], msgk[:], AF.Ln, bias=epsk_sb[:],
                                     scale=1.0 / H)
                nc.scalar.activation(rsq[:], msgq[:], AF.Ln, bias=epsq_sb[:])
                nc.scalar.activation(rsk[:], rsk[:], AF.Exp, scale=-0.5)
                nc.scalar.activation(rsq[:], rsq[:], AF.Exp, scale=-0.5)
                for i in range(TS):
                    for hb in range(HB):
                        col = slice(i * HB + hb, i * HB + hb + 1)
                        nc.vector.tensor_scalar_mul(
                            knG[:, i, hb, :], knG[:, i, hb, :], rsk[:, col])
                        nc.vector.tensor_scalar_mul(
                            qnG[:, i, hb, :], qnG[:, i, hb, :], rsq[:, col])

            def load_x(tt):
                x_tile = xpool.tile([P, CT, P], BF16, tag="xs", name=f"x{tt}")
                nc.sync.dma_start(
                    x_tile[:], xh[tt].rearrange("p (ct t) -> p ct t", ct=CT))
                return x_tile

            # ======== phase 0: K, V, Q for all tokens ========
            for gg in range(NTT // TS):
                knG = qpool.tile([P, TS, HB, H], BF16, tag="kn", name=f"kn{gg}")
                qnG = qpool.tile([P, TS, HB, H], BF16, tag="qn", name=f"qn{gg}")
                msgk = wk_pool.tile([P, HB * TS], F32, tag="mgk", name=f"mk{gg}")
                msgq = wk_pool.tile([P, HB * TS], F32, tag="mgq", name=f"mq{gg}")
                x_tiles = [load_x(gg * TS + i) for i in range(TS)]
                for i in range(TS):
                    tt = gg * TS + i
                    pk = proj(x_tiles[i], wk_sb, f"pk{tt}")
                    rope(tt, i, pk, knG, kw_sb if apply_w else None, msgk)
                for i in range(TS):
                    tt = gg * TS + i
                    pv = proj(x_tiles[i], wv_sb, f"pv{tt}")
                    nc.vector.tensor_copy(v_sb[:, tt, :, 0:H], pv[:])
                for i in range(TS):
                    tt = gg * TS + i
                    pq = proj(x_tiles[i], wq_sb, f"pq{tt}")
                    rope(tt, i, pq, qnG, qw_sb if apply_w else None, msgq)
                norm_scale(knG, qnG, msgk, msgq, gg)
                # batched K transpose: [tok,(i hb h)] -> [h,(kt hb),tok]
                nc.sync.dma_start_transpose(
                    kT_sb[:, gg * TS:(gg + 1) * TS, :, :].rearrange(
                        "p i hb q -> p (i hb) q"),
                    knG[:].rearrange("p i hb h -> p (i hb h)"))
                # per-tile Q transpose: [tok,(hb h)] -> [h, hb, tok]
                for i in range(TS):
                    nc.sync.dma_start_transpose(
                        qT_sb[:, :, gg * TS + i, :],
                        qnG[:, i].rearrange("p hb h -> p (hb h)"))

            # ======== phase 1: attention + Wo + RS per query chunk ========
            for qc in range(NQC):
                a_sb = opool.tile([P, TS, HB * H], BF16, tag="a",
                                  name=f"a{qc}")      # [tq, ts, (hb h)]
                nkt = (qc + 1) * TS
                for hp in range(2):
                    hbs = (2 * hp, 2 * hp + 1)
                    o_ps = {}
                    for hb in hbs:
                        for j in range(2):
                            o = psO.tile([P, 2, 130], F32, tag="o",
                                         name=f"o_{qc}_{hb}_{j}")
                            nc.tensor.matmul(
                                o[:].rearrange("p a b -> p (a b)"),
                                zrow_sb[:, 0:P], zrow_sb[:],
                                start=True, stop=False)
                            o_ps[(hb, j)] = o

                    def emit_pv(kt, hb, p_sb):
                        d = kt - qc * TS
                        for ts in range(max(d, 0), TS):
                            nc.tensor.matmul(
                                o_ps[(hb, ts // 2)][:, ts % 2, 0:H + 1],
                                p_sb[:, ts * P:(ts + 1) * P],
                                v_sb[:, kt, hb, :],
                                start=False,
                                stop=(kt == qc * TS + ts))

                    pend = []
                    for kt in range(nkt):
                        d = kt - qc * TS
                        lo = max(d, 0) * P
                        for hb in hbs:
                            sp = psS if hb == hbs[0] else psA
                            stag = "tp" if hb == hbs[0] else "proj"
                            s_ps = sp.tile([P, QC], F32, tag=stag,
                                           name=f"s_{qc}_{hb}_{kt}")
                            nc.tensor.matmul(
                                s_ps[:, lo:], kT_sb[:, kt, hb, :],
                                qT_sb[:, hb, qc * TS:(qc + 1) * TS, :]
                                .rearrange("p ts t -> p (ts t)")[:, lo:],
                                start=True, stop=True)
                            p_sb = ppool.tile([P, QC], BF16, tag="p",
                                              name=f"p_{qc}_{hb}_{kt}")
                            nc.scalar.activation(p_sb[:, lo:], s_ps[:, lo:],
                                                 AF.Exp)
                            if d >= 0:
                                nc.vector.tensor_mul(
                                    p_sb[:, d * P:(d + 1) * P],
                                    p_sb[:, d * P:(d + 1) * P],
                                    mask_sb[:])
                            pend.append((kt, hb, p_sb))
                        # software pipeline: PV for kt-1 runs while exp(kt)
                        # is still on the scalar engine
                        while pend and pend[0][0] < kt:
                            emit_pv(*pend.pop(0))
                    for item in pend:
                        emit_pv(*item)
                    for hb in hbs:
                        for ts in range(TS):
                            o = o_ps[(hb, ts // 2)]
                            den = wk_pool.tile([P, 1], F32, tag="den",
                                               name=f"dn{qc}_{hb}_{ts}")
                            nc.vector.reciprocal(den[:], o[:, ts % 2, H:H + 1])
                            nc.vector.tensor_scalar_mul(
                                a_sb[:, ts, hb * H:(hb + 1) * H],
                                o[:, ts % 2, 0:H], den[:])

                # A transpose on PE (DMA transposes serialize with
                # collectives, stalling the pipeline on the previous RS)
                aT_sb = opool.tile([P, TS, HB, P], BF16, tag="aT",
                                   name=f"aT{qc}")    # [h, ts, fb, tq]
                for ts in range(TS):
                    tp = psS.tile([P, HB, P], BF16, tag="tp",
                                  name=f"tp{qc}_{ts}")
                    for fb in range(HB):
                        nc.tensor.transpose(
                            tp[:, fb, :], a_sb[:, ts, fb * P:(fb + 1) * P],
                            ident[:])
                    nc.vector.tensor_copy(aT_sb[:, ts], tp[:])

                # Wo partials -> bf16 bounce -> ReduceScatter
                bounce = ccpool.tile([QC, C], BF16, tag=f"bounce{qc}",
                                     name=f"bounce{qc}")
                for ts in range(TS):
                    obG = opool.tile([P, C], BF16, tag="ob", name=f"ob{qc}_{ts}")
                    for cc in range(C // QC):
                        wo_ps = psA.tile([P, QC], F32, tag="proj",
                                         name=f"wops{qc}_{ts}_{cc}")
                        for fb in range(HB):
                            nc.tensor.matmul(
                                wo_ps[:], aT_sb[:, ts, fb, :],
                                wo_sb[:, fb, cc * QC:(cc + 1) * QC],
                                start=(fb == 0), stop=(fb == HB - 1))
                        nc.vector.tensor_copy(obG[:, cc * QC:(cc + 1) * QC],
                                               wo_ps[:])
                    nc.sync.dma_start(
                        bounce[ts * P:(ts + 1) * P, :], obG[:])
                if qc == NQC - 1:
                    for s in range(2):
                        red = ccpool.tile([P // 2, C], BF16, tag=f"redL{s}",
                                          name=f"redL{s}")
                        nc.gpsimd.collective_compute(
                            "ReduceScatter",
                            ALU.add,
                            ins=[bounce[s * (QC // 2):(s + 1) * (QC // 2), :].opt()],
                            outs=[red[:].opt()],
                            replica_groups=[[0, 1, 2, 3], [4, 5, 6, 7]],
                        )
                        nc.gpsimd.dma_start(
                            out_e[qc, s * (P // 2):(s + 1) * (P // 2), :],
                            red[:])
                else:
                    red = ccpool.tile([P, C], BF16, tag=f"red{qc}",
                                      name=f"red{qc}")
                    nc.gpsimd.collective_compute(
                        "ReduceScatter",
                        ALU.add,
                        ins=[bounce[:].opt()],
                        outs=[red[:].opt()],
                        replica_groups=[[0, 1, 2, 3], [4, 5, 6, 7]],
                    )
                    nc.gpsimd.dma_start(out_e[qc], red[:])

    nc.compile()
    return nc


def _get_nc(apply_w):
    key = ("nc", apply_w)
    if key not in _NC_CACHE:
        _NC_CACHE[key] = build_nc(apply_w)
    return _NC_CACHE[key]


def make_in_maps(x, sin, cos, Wq, Wk, Wv, Wo, q_norm_w, k_norm_w):
    bf = ml_dtypes.bfloat16

    def part_major(wT):
        # [C, N] -> [P, CT*N]: row p holds [wT[ct*128+p, :] for ct]
        Cdim, N = wT.shape
        return np.ascontiguousarray(
            wT.reshape(Cdim // P, P, N).transpose(1, 0, 2).reshape(P, -1))

    cos_f = part_major(np.asarray(cos, np.float32).astype(bf))
    sin_f = part_major(np.asarray(sin, np.float32).astype(bf))
    mask = (np.arange(P)[:, None] <= np.arange(P)[None, :]).astype(bf)
    qw = np.tile(np.asarray(q_norm_w, np.float32)[None, :], (P, HB)).astype(bf)
    kw = np.tile(np.asarray(k_norm_w, np.float32)[None, :], (P, HB)).astype(bf)
    in_maps = []
    for i in range(8):
        b, g = divmod(i, G)
        sl = slice(g * HB * H, (g + 1) * HB * H)
        xT = np.asarray(x[b], np.float32).T.astype(bf)   # [C, T]
        # xh[tt, p, ct*128+c] = xT[ct*128+p, tt*128+c]
        xh = np.ascontiguousarray(
            xT.reshape(CT, P, NTT, P).transpose(2, 1, 0, 3).reshape(NTT, P, CT * P))
        in_maps.append({
            "xh": xh,
            "wq": part_major(np.asarray(Wq, np.float32)[sl, :].T.astype(bf)),
            "wk": part_major(np.asarray(Wk, np.float32)[sl, :].T.astype(bf)),
            "wv": part_major(np.asarray(Wv, np.float32)[sl, :].T.astype(bf)),
            "wo": part_major(np.asarray(Wo, np.float32)[:, sl].T.astype(bf)),
            "cos": cos_f, "sin": sin_f, "mask": mask, "qw": qw, "kw": kw,
        })
    return in_maps


def assemble_output(results):
    out = np.empty((B, T, C), np.float32)
    W2 = P // 2  # 64-row sub-shards from the split last-chunk RS
    for i in range(8):
        b, g = divmod(i, G)
        r = results[i]["out"]  # [NQC, P, C] bf16
        for qc in range(NQC - 1):
            t0 = qc * QC + g * P
            out[b, t0:t0 + P, :] = r[qc].astype(np.float32)
        qc = NQC - 1
        for s in range(2):
            t0 = qc * QC + s * (QC // 2) + g * W2
            out[b, t0:t0 + W2, :] = r[qc][s * W2:(s + 1) * W2].astype(np.float32)
    return out


def kernel(x, sin, cos, Wq, Wk, Wv, Wo, q_norm_w, k_norm_w):
    apply_w = not (np.allclose(np.asarray(q_norm_w), 1.0)
                   and np.allclose(np.asarray(k_norm_w), 1.0))
    nc = _get_nc(apply_w)
    in_maps = make_in_maps(x, sin, cos, Wq, Wk, Wv, Wo, q_norm_w, k_norm_w)
    res = run_bass_kernel_spmd(nc, in_maps, core_ids=list(range(8)))
    return assemble_output(res.results)
